# revision 5
# baseline (speedup 1.0000x reference)
"""Trainium2 Bass kernel for BertWithAdaThresholdLocContextPooling.

Strategy: pure data parallel over batch (B=16 -> 2 batches per core x 8 cores).
Each core:
  - gathers mention rows of sequence_output / attention via indirect DMA
    (only ~0.4MB of the 25MB attention shard is ever read from HBM),
  - logsumexp-pools mention embeddings, mean-pools attention rows,
  - computes the localized-context attention rs = seq^T @ ht,
  - runs the two extractor GEMVs (bf16 weights, fp32 accumulate),
  - forms the grouped bilinear via PE replication matmuls,
  - applies the classifier Wb.
Weights are replicated to all cores; host pre-transposes/casts them.
"""

import sys

for _p in ("/opt/trn_rl_repo",):
    if _p not in sys.path:
        sys.path.insert(0, _p)

import numpy as np
import ml_dtypes

import concourse.bacc as bacc
import concourse.bass as bass
import concourse.mybir as mybir
from concourse.tile import TileContext
from concourse.bass_utils import run_bass_kernel_spmd

F32 = mybir.dt.float32
F32R = mybir.dt.float32r
BF16 = mybir.dt.bfloat16
I32 = mybir.dt.int32
AF = mybir.ActivationFunctionType
ALU = mybir.AluOpType

B, L, HID = 16, 512, 768
HEADS, M = 12, 4
EMB, BLK, NER, NCLS = 768, 8, 6, 97
NCORES = 8
BPC = B // NCORES          # batches per core = 2
CAT = 2 * HID + NER        # 1542
KCH = 12                   # full 128-row contraction chunks of CAT
NEMB = EMB // 128          # 6 chunks of EMB
NL = L // 128              # 4 chunks of L
NBL = EMB * BLK // 128     # 48 classifier contraction chunks

_cache = {}


def _build_constants():
    c = {}
    # entity-sum selection: partition k=(b,i,m) -> col j=(b,i)
    selE = np.zeros((4 * M, 4), np.float32)
    for k in range(4 * M):
        selE[k, k // M] = 1.0
    c["selE"] = selE
    # attention combine: partition k=(i,m,h) -> col (i,h), mean over m
    selA = np.zeros((2 * M * HEADS, 2 * HEADS), np.float32)
    for i in range(2):
        for m in range(M):
            for h in range(HEADS):
                selA[i * M * HEADS + m * HEADS + h, i * HEADS + h] = 1.0 / M
    c["selA"] = selA
    # head-mean column
    c["w12"] = np.full((HEADS, 1), 1.0 / HEADS, np.float32)
    # replicate 8 mention positions (i,m) -> 96 rows (i,m,h)
    rep8 = np.zeros((2 * M, 2 * M * HEADS), np.float32)
    for q in range(2 * M * HEADS):
        rep8[q // HEADS, q] = 1.0
    c["rep8"] = rep8
    # attention row-index base per (q=(i,m,h), local batch b): (b*HEADS+h)*L + 1
    baseA = np.zeros((2 * M * HEADS, BPC), np.float32)
    for q in range(2 * M * HEADS):
        h = q % HEADS
        for b in range(BPC):
            baseA[q, b] = (b * HEADS + h) * L + 1
    c["baseA"] = baseA
    # sequence row-index base per k=(b,i,m): b*L + 1
    baseS = np.zeros((4 * M, 1), np.float32)
    for k in range(4 * M):
        baseS[k, 0] = (k // (2 * M)) * L + 1
    c["baseS"] = baseS
    c["ident"] = np.eye(128, dtype=np.float32)
    # bilinear ts-replication selectors: col p of RY_y maps to row (p//8)*8+y
    rys = np.zeros((128, 8 * 128), ml_dtypes.bfloat16)
    for y in range(BLK):
        for p in range(128):
            rys[(p // BLK) * BLK + y, y * 128 + p] = 1.0
    c["rys"] = rys
    # bias-injection row selectors (rows of the 4-col cat block)
    c["selbh"] = np.array([[1.0, 0.0, 1.0, 0.0]], ml_dtypes.bfloat16)
    c["selbt"] = np.array([[0.0, 1.0, 0.0, 1.0]], ml_dtypes.bfloat16)
    # classifier row permutation: chunk k=(c,y), partition p -> g=(c*16+p//8), x=p%8
    perm = np.empty(EMB * BLK, np.int64)
    for cch in range(NEMB):
        for y in range(BLK):
            for p in range(128):
                g = cch * 16 + p // BLK
                x = p % BLK
                perm[(cch * BLK + y) * 128 + p] = g * 64 + x * BLK + y
    c["perm"] = perm
    return c


def _build_program():
    nc = bacc.Bacc("TRN2", target_bir_lowering=False, debug=False)

    seq_h = nc.dram_tensor("seq", [BPC * L, HID], F32, kind="ExternalInput")
    attn_h = nc.dram_tensor("attn", [BPC * HEADS * L, L], F32, kind="ExternalInput")
    pos_h = nc.dram_tensor("pos", [4 * M, 1], I32, kind="ExternalInput")
    posb_hs = [
        nc.dram_tensor(f"posb{b}", [2 * M, 1], I32, kind="ExternalInput")
        for b in range(BPC)
    ]
    nh_h = nc.dram_tensor("nh", [BPC, NER], F32, kind="ExternalInput")
    nt_h = nc.dram_tensor("nt", [BPC, NER], F32, kind="ExternalInput")
    whT_h = nc.dram_tensor("whT", [CAT, EMB], BF16, kind="ExternalInput")
    wtT_h = nc.dram_tensor("wtT", [CAT, EMB], BF16, kind="ExternalInput")
    wbT_h = nc.dram_tensor("wbT", [EMB * BLK, 128], BF16, kind="ExternalInput")
    bhr_h = nc.dram_tensor("bhr", [1, EMB], BF16, kind="ExternalInput")
    btr_h = nc.dram_tensor("btr", [1, EMB], BF16, kind="ExternalInput")
    bbc_h = nc.dram_tensor("bbc", [NCLS, 1], F32, kind="ExternalInput")
    selE_h = nc.dram_tensor("selE", [4 * M, 4], F32, kind="ExternalInput")
    selA_h = nc.dram_tensor("selA", [2 * M * HEADS, 2 * HEADS], F32, kind="ExternalInput")
    w12_h = nc.dram_tensor("w12", [HEADS, 1], F32, kind="ExternalInput")
    rep8_h = nc.dram_tensor("rep8", [2 * M, 2 * M * HEADS], F32, kind="ExternalInput")
    baseA_h = nc.dram_tensor("baseA", [2 * M * HEADS, BPC], F32, kind="ExternalInput")
    baseS_h = nc.dram_tensor("baseS", [4 * M, 1], F32, kind="ExternalInput")
    ident_h = nc.dram_tensor("ident", [128, 128], F32, kind="ExternalInput")
    rys_h = nc.dram_tensor("rys", [128, 8 * 128], BF16, kind="ExternalInput")
    selbh_h = nc.dram_tensor("selbh", [1, 4], BF16, kind="ExternalInput")
    selbt_h = nc.dram_tensor("selbt", [1, 4], BF16, kind="ExternalInput")
    out_h = nc.dram_tensor("logitsT", [NCLS, BPC], F32, kind="ExternalOutput")

    with TileContext(nc) as tc:
        with (
            tc.tile_pool(name="const", bufs=1) as cp,
            tc.tile_pool(name="data", bufs=1) as dp,
            tc.tile_pool(name="wch", bufs=4) as wp,
            tc.tile_pool(name="wbch", bufs=4) as wbp,
            tc.tile_pool(name="psbig", bufs=1, space="PSUM") as psb,
            tc.tile_pool(name="psea", bufs=2, space="PSUM") as pse,
            tc.tile_pool(name="pssm", bufs=3, space="PSUM") as pss,
        ):
            # ---- constant loads ----
            selE = cp.tile([4 * M, 4], F32)
            nc.sync.dma_start(selE[:], selE_h[:])
            selA = cp.tile([2 * M * HEADS, 2 * HEADS], F32)
            nc.sync.dma_start(selA[:], selA_h[:])
            w12 = cp.tile([HEADS, 1], F32)
            nc.sync.dma_start(w12[:], w12_h[:])
            rep8 = cp.tile([2 * M, 2 * M * HEADS], F32)
            nc.sync.dma_start(rep8[:], rep8_h[:])
            baseA = cp.tile([2 * M * HEADS, BPC], F32)
            nc.sync.dma_start(baseA[:], baseA_h[:])
            baseS = cp.tile([4 * M, 1], F32)
            nc.sync.dma_start(baseS[:], baseS_h[:])
            ident = cp.tile([128, 128], F32)
            nc.sync.dma_start(ident[:], ident_h[:])
            rys = cp.tile([128, 8 * 128], BF16)
            nc.sync.dma_start(rys[:], rys_h[:])
            selbh = cp.tile([1, 4], BF16)
            nc.sync.dma_start(selbh[:], selbh_h[:])
            selbt = cp.tile([1, 4], BF16)
            nc.sync.dma_start(selbt[:], selbt_h[:])
            bhr = cp.tile([1, EMB], BF16)
            nc.sync.dma_start(bhr[:], bhr_h[:])
            btr = cp.tile([1, EMB], BF16)
            nc.sync.dma_start(btr[:], btr_h[:])
            bbc = cp.tile([NCLS, 1], F32)
            nc.sync.dma_start(bbc[:], bbc_h[:])

            # ---- index computation ----
            posi = dp.tile([4 * M, 1], I32)
            nc.sync.dma_start(posi[:], pos_h[:])
            posf = dp.tile([4 * M, 1], F32)
            nc.vector.tensor_copy(posf[:], posi[:])
            idxsf = dp.tile([4 * M, 1], F32)
            nc.vector.tensor_add(idxsf[:], posf[:], baseS[:])
            idxs = dp.tile([4 * M, 1], I32)
            nc.vector.tensor_copy(idxs[:], idxsf[:])

            idxa = []
            for b in range(BPC):
                posbi = dp.tile([2 * M, 1], I32, tag=f"posbi{b}")
                nc.sync.dma_start(posbi[:], posb_hs[b][:])
                posbf = dp.tile([2 * M, 1], F32, tag=f"posbf{b}")
                nc.vector.tensor_copy(posbf[:], posbi[:])
                ps_idx = pss.tile([2 * M * HEADS, 1], F32, tag="sm")
                nc.tensor.matmul(ps_idx[:], lhsT=rep8[:], rhs=posbf[:],
                                 start=True, stop=True)
                idxaf = dp.tile([2 * M * HEADS, 1], F32, tag=f"idxaf{b}")
                nc.vector.tensor_add(idxaf[:], ps_idx[:], baseA[:, b:b + 1])
                ia = dp.tile([2 * M * HEADS, 1], I32, tag=f"idxa{b}")
                nc.vector.tensor_copy(ia[:], idxaf[:])
                idxa.append(ia)

            # ---- gathers ----
            sg = dp.tile([4 * M, HID], F32)
            nc.gpsimd.indirect_dma_start(
                out=sg[:], out_offset=None, in_=seq_h[:],
                in_offset=bass.IndirectOffsetOnAxis(ap=idxs[:, :1], axis=0))
            at = []
            for b in range(BPC):
                t = dp.tile([2 * M * HEADS, L], F32, tag=f"at{b}")
                nc.gpsimd.indirect_dma_start(
                    out=t[:], out_offset=None, in_=attn_h[:],
                    in_offset=bass.IndirectOffsetOnAxis(ap=idxa[b][:, :1], axis=0))
                at.append(t)

            # ---- entity embeddings: log-sum-exp over mentions ----
            exps = dp.tile([4 * M, HID], F32)
            nc.scalar.activation(exps[:], sg[:], AF.Exp)
            ps_e = psb.tile([4, HID], F32, tag="big")
            for n0, nl_ in ((0, 512), (512, 256)):
                nc.tensor.matmul(ps_e[:, n0:n0 + nl_], lhsT=selE[:],
                                 rhs=exps[:, n0:n0 + nl_], start=True, stop=True)
            ent = dp.tile([4, HID], F32)
            nc.scalar.activation(ent[:], ps_e[:], AF.Ln)
            # transpose to columns: entT[:, c*4+r] = ent[r, c*128+p]
            ps_et = pss.tile([128, 4 * NEMB], F32, tag="sm")
            for c in range(NEMB):
                nc.tensor.transpose(ps_et[:, c * 4:(c + 1) * 4],
                                    ent[:, c * 128:(c + 1) * 128], ident[0:4, 0:4])
            entT = dp.tile([128, 4 * NEMB], BF16)
            nc.vector.tensor_copy(entT[:], ps_et[:])

            # ---- entity attention pooling + context vector ----
            htc = []
            for b in range(BPC):
                ps_eah = pse.tile([HEADS, L], F32, tag="ea")
                nc.tensor.matmul(ps_eah[:], lhsT=selA[:, 0:HEADS], rhs=at[b][:],
                                 start=True, stop=True)
                ps_eat = pse.tile([HEADS, L], F32, tag="ea")
                nc.tensor.matmul(ps_eat[:], lhsT=selA[:, HEADS:2 * HEADS], rhs=at[b][:],
                                 start=True, stop=True)
                eah = dp.tile([HEADS, L], F32, tag=f"eah{b}")
                nc.vector.tensor_copy(eah[:], ps_eah[:])
                prd = dp.tile([HEADS, L], F32, tag=f"prd{b}")
                nc.vector.tensor_tensor(out=prd[:], in0=eah[:], in1=ps_eat[:],
                                        op=ALU.mult)
                ps_ht = pss.tile([1, L], F32, tag="sm")
                nc.tensor.matmul(ps_ht[:], lhsT=w12[:], rhs=prd[:],
                                 start=True, stop=True)
                sm = dp.tile([1, 1], F32, tag=f"sm{b}")
                nc.vector.reduce_sum(sm[:], ps_ht[:], axis=mybir.AxisListType.X)
                den = dp.tile([1, 1], F32, tag=f"den{b}")
                nc.vector.tensor_scalar_add(den[:], sm[:], 1e-5)
                rcp = dp.tile([1, 1], F32, tag=f"rcp{b}")
                nc.vector.reciprocal(rcp[:], den[:])
                htn = dp.tile([1, L], F32, tag=f"htn{b}")
                nc.vector.tensor_scalar_mul(htn[:], ps_ht[:], rcp[:, :1])
                ps_htc = pss.tile([128, NL], F32, tag="sm")
                for c in range(NL):
                    nc.tensor.transpose(ps_htc[:, c:c + 1],
                                        htn[:, c * 128:(c + 1) * 128],
                                        ident[0:1, 0:1])
                h = dp.tile([128, NL], F32, tag=f"htc{b}")
                nc.vector.tensor_copy(h[:], ps_htc[:])
                htc.append(h)

            # ---- rs = seq^T @ ht  (column form) ----
            seqt = []
            for b in range(BPC):
                t = dp.tile([128, NL * HID], F32, tag=f"seq{b}")
                nc.sync.dma_start(
                    t[:].rearrange("p (c d) -> p c d", c=NL),
                    seq_h[b * L:(b + 1) * L, :].rearrange("(c p) d -> p c d", p=128))
                seqt.append(t)
            ps_rsc = pss.tile([128, NEMB * BPC], F32, tag="sm")
            for b in range(BPC):
                for d in range(NEMB):
                    for c in range(NL):
                        nc.tensor.matmul(
                            ps_rsc[:, d * BPC + b:d * BPC + b + 1],
                            lhsT=seqt[b][:, c * HID + d * 128:c * HID + (d + 1) * 128],
                            rhs=htc[b][:, c:c + 1],
                            start=(c == 0), stop=(c == NL - 1))
            # duplicate into cat-column layout (b0,b0,b1,b1) per chunk
            rsc = dp.tile([128, 4 * NEMB], BF16)
            nc.vector.tensor_copy(
                rsc[:].rearrange("p (r b m) -> p r b m", r=NEMB, b=BPC),
                ps_rsc[:].rearrange("p (r b) -> p r b", r=NEMB)
                .unsqueeze(3).broadcast_to([128, NEMB, BPC, 2]))

            # ---- ner tag columns ----
            ner4f = dp.tile([NER, 4], F32)
            for b in range(BPC):
                nc.sync.dma_start(ner4f[:, 2 * b:2 * b + 1],
                                  nh_h[b:b + 1, :].rearrange("a e -> e a"))
                nc.sync.dma_start(ner4f[:, 2 * b + 1:2 * b + 2],
                                  nt_h[b:b + 1, :].rearrange("a e -> e a"))
            ner4 = dp.tile([NER, 4], BF16)
            nc.vector.tensor_copy(ner4[:], ner4f[:])

            # ---- extractor GEMVs:  [4,768] = cat4^T @ W^T  ----
            def cat_chunk(j):
                if j < NEMB:
                    return entT[:, j * 4:(j + 1) * 4]
                if j < 2 * NEMB:
                    return rsc[:, (j - NEMB) * 4:(j - NEMB + 1) * 4]
                return ner4[:]

            t4 = []
            for wi, (w_h, selb, br) in enumerate(
                    ((whT_h, selbh, bhr), (wtT_h, selbt, btr))):
                ps_w = psb.tile([4, EMB], F32, tag="big")
                for j in range(KCH + 1):
                    k0 = j * 128
                    klen = 128 if j < KCH else NER
                    wch = wp.tile([128, EMB], BF16, tag="wch")
                    nc.sync.dma_start(wch[:klen, :], w_h[k0:k0 + klen, :])
                    for n0, nl_ in ((0, 512), (512, 256)):
                        nc.tensor.matmul(ps_w[:, n0:n0 + nl_], lhsT=cat_chunk(j),
                                         rhs=wch[:klen, n0:n0 + nl_],
                                         start=(j == 0), stop=False)
                # bias via rank-1 update, then tanh
                for n0, nl_ in ((0, 512), (512, 256)):
                    nc.tensor.matmul(ps_w[:, n0:n0 + nl_], lhsT=selb[:],
                                     rhs=br[:, n0:n0 + nl_], start=False, stop=True)
                t = dp.tile([4, EMB], F32, tag=f"t4_{wi}")
                nc.scalar.activation(t[:], ps_w[:], AF.Tanh)
                t4.append(t)

            # ---- transpose hs2/ts2 to columns ----
            ps_a = pss.tile([128, 4 * NEMB], F32, tag="sm")
            ps_b = pss.tile([128, 4 * NEMB], F32, tag="sm")
            for c in range(NEMB):
                nc.tensor.transpose(ps_a[:, c * 4:(c + 1) * 4],
                                    t4[0][:, c * 128:(c + 1) * 128], ident[0:4, 0:4])
                nc.tensor.transpose(ps_b[:, c * 4:(c + 1) * 4],
                                    t4[1][:, c * 128:(c + 1) * 128], ident[0:4, 0:4])
            h2t = dp.tile([128, 4 * NEMB], BF16)
            nc.vector.tensor_copy(
                h2t[:].rearrange("p (c b) -> p c b", c=NEMB)[:, :, 0:4:2],
                ps_a[:].rearrange("p (c b) -> p c b", c=NEMB)[:, :, 0:4:2])
            nc.vector.tensor_copy(
                h2t[:].rearrange("p (c b) -> p c b", c=NEMB)[:, :, 1:4:2],
                ps_b[:].rearrange("p (c b) -> p c b", c=NEMB)[:, :, 1:4:2])

            # ---- grouped bilinear + classifier ----
            ps_t2x = pss.tile([128, NEMB * 16], F32, tag="sm")
            for y in range(BLK):
                for c in range(NEMB):
                    nc.tensor.matmul(
                        ps_t2x[:, c * 16 + y * 2:c * 16 + y * 2 + 2],
                        lhsT=rys[:, y * 128:(y + 1) * 128],
                        rhs=h2t[:, c * 4 + 1:c * 4 + 4:2],
                        start=True, stop=True)
            blt = dp.tile([128, NEMB * 16], BF16)
            for c in range(NEMB):
                nc.vector.tensor_tensor(
                    out=blt[:, c * 16:(c + 1) * 16].rearrange("p (y b) -> p y b", y=BLK),
                    in0=h2t[:, c * 4:c * 4 + 4:2].unsqueeze(1)
                        .broadcast_to([128, BLK, 2]),
                    in1=ps_t2x[:, c * 16:(c + 1) * 16].rearrange("p (y b) -> p y b", y=BLK),
                    op=ALU.mult)
            ps_l = pss.tile([NCLS, BPC], F32, tag="sm")
            for c in range(NEMB):
                for y in range(BLK):
                    k = c * BLK + y
                    wbch = wbp.tile([128, 128], BF16, tag="wbch")
                    nc.sync.dma_start(wbch[:], wbT_h[k * 128:(k + 1) * 128, :])
                    nc.tensor.matmul(ps_l[:], lhsT=wbch[:, 0:NCLS],
                                     rhs=blt[:, c * 16 + y * 2:c * 16 + y * 2 + 2],
                                     start=(k == 0), stop=(k == NBL - 1))
            lg = dp.tile([NCLS, BPC], F32)
            nc.vector.tensor_scalar_add(lg[:], ps_l[:], bbc[:, :1])
            nc.sync.dma_start(out_h[:], lg[:])

    nc.finalize()
    return nc


def _get_program():
    if "nc" not in _cache:
        _cache["nc"] = _build_program()
        _cache["consts"] = _build_constants()
    return _cache["nc"], _cache["consts"]


def kernel(sequence_output, attention, entity_pos, hs_ner_tags, ts_ner_tags,
           Wh, bh, Wt, bt, Wb, bb):
    nc, c = _get_program()

    seq = np.ascontiguousarray(np.asarray(sequence_output, dtype=np.float32))
    attn = np.ascontiguousarray(np.asarray(attention, dtype=np.float32))
    pos = np.asarray(entity_pos).astype(np.int32)
    nh = np.ascontiguousarray(np.asarray(hs_ner_tags, dtype=np.float32))
    nt = np.ascontiguousarray(np.asarray(ts_ner_tags, dtype=np.float32))
    whT = np.ascontiguousarray(np.asarray(Wh, dtype=np.float32).T).astype(ml_dtypes.bfloat16)
    wtT = np.ascontiguousarray(np.asarray(Wt, dtype=np.float32).T).astype(ml_dtypes.bfloat16)
    wbT = np.ascontiguousarray(np.asarray(Wb, dtype=np.float32).T)[c["perm"]]
    wbT = np.pad(wbT, ((0, 0), (0, 128 - NCLS))).astype(ml_dtypes.bfloat16)
    bhr = np.asarray(bh, dtype=np.float32).reshape(1, EMB).astype(ml_dtypes.bfloat16)
    btr = np.asarray(bt, dtype=np.float32).reshape(1, EMB).astype(ml_dtypes.bfloat16)
    bbc = np.asarray(bb, dtype=np.float32).reshape(NCLS, 1)

    in_maps = []
    for core in range(NCORES):
        b0 = core * BPC
        pc = np.ascontiguousarray(pos[b0:b0 + BPC])          # [2,2,M]
        im = {
            "seq": seq[b0:b0 + BPC].reshape(BPC * L, HID),
            "attn": attn[b0:b0 + BPC].reshape(BPC * HEADS * L, L),
            "pos": pc.reshape(4 * M, 1),
            "nh": nh[b0:b0 + BPC],
            "nt": nt[b0:b0 + BPC],
            "whT": whT, "wtT": wtT, "wbT": wbT,
            "bhr": bhr, "btr": btr, "bbc": bbc,
            "selE": c["selE"], "selA": c["selA"], "w12": c["w12"],
            "rep8": c["rep8"], "baseA": c["baseA"], "baseS": c["baseS"],
            "ident": c["ident"], "rys": c["rys"],
            "selbh": c["selbh"], "selbt": c["selbt"],
        }
        for b in range(BPC):
            im[f"posb{b}"] = np.ascontiguousarray(pc[b].reshape(2 * M, 1))
        in_maps.append(im)

    res = run_bass_kernel_spmd(nc, in_maps, core_ids=list(range(NCORES)))
    _cache["last_res"] = res
    out = np.empty((B, NCLS), np.float32)
    for core in range(NCORES):
        out[core * BPC:(core + 1) * BPC] = res.results[core]["logitsT"].T
    return out


# revision 15
# speedup vs baseline: 1.6573x; 1.6573x over previous
"""Trainium2 Bass kernel for BertWithAdaThresholdLocContextPooling.

Strategy: pure data parallel over batch (B=16 -> 2 batches per core x 8 cores).
Each core:
  - gathers mention rows of sequence_output / attention via indirect DMA
    (only ~0.2MB of the 12.6MB attention shard is ever read from HBM),
  - logsumexp-pools mention embeddings, mean-pools attention rows,
  - computes the localized-context attention rs = seq^T @ ht,
  - runs the two extractor GEMVs (bf16 data, fp32 accumulate),
  - forms the grouped bilinear via PE replication matmuls,
  - applies the classifier Wb.
Weights are replicated to all cores; the host pre-transposes/casts them and
packs small constants so each core issues only a handful of large DMAs.
"""

import sys

for _p in ("/opt/trn_rl_repo",):
    if _p not in sys.path:
        sys.path.insert(0, _p)

import numpy as np
import ml_dtypes

import concourse.bacc as bacc
import concourse.bass as bass
import concourse.mybir as mybir
from concourse.tile import TileContext
from concourse.bass_utils import run_bass_kernel_spmd

F32 = mybir.dt.float32
BF16 = mybir.dt.bfloat16
I32 = mybir.dt.int32
AF = mybir.ActivationFunctionType
ALU = mybir.AluOpType

B, L, HID = 16, 512, 768
HEADS, M = 12, 4
EMB, BLK, NER, NCLS = 768, 8, 6, 97
NCORES = 8
BPC = B // NCORES          # batches per core = 2
CAT = 2 * HID + NER        # 1542
KCH = 12                   # full 128-row contraction chunks of CAT
NEMB = EMB // 128          # 6 chunks of EMB
NL = L // 128              # 4 chunks of L
NBL = EMB * BLK // 128     # 48 classifier contraction chunks

# packed-constant layouts
# CF32 [128, 228]: rep8 [0:8,0:96] | baseA [0:96,96:98] | baseS [0:16,98:99]
#                  | bbc [0:97,99:100] | identity f32 [0:128,100:228]
IDF0 = 100
CF32_COLS = 228
# CB16 [128, 1924]: rys [0:128,0:1024] | selE [0:16,1024:1028]
#   | selA [0:96,1028:1052] | w12 [0:12,1052:1053] | bhr [0:1,1053:1821->no]
# (bhr/btr need legal matmul row bases: bhr row 0, btr row 32)
RYS0 = 0
SELE0 = 1024
SELA0 = 1028
W120 = 1052
BHR0 = 1056
BTR0 = BHR0 + 768
SELBH0 = BTR0 + 768
CB16_COLS = SELBH0 + 8

_cache = {}


def _build_constants():
    selE = np.zeros((4 * M, 4), np.float32)
    for k in range(4 * M):
        selE[k, k // M] = 1.0
    selA = np.zeros((2 * M * HEADS, 2 * HEADS), np.float32)
    for i in range(2):
        for m in range(M):
            for h in range(HEADS):
                selA[i * M * HEADS + m * HEADS + h, i * HEADS + h] = 1.0 / M
    rep8 = np.zeros((2 * M, 2 * M * HEADS), np.float32)
    for q in range(2 * M * HEADS):
        rep8[q // HEADS, q] = 1.0
    baseA = np.zeros((2 * M * HEADS, BPC), np.float32)
    for q in range(2 * M * HEADS):
        for b in range(BPC):
            baseA[q, b] = (b * HEADS + q % HEADS) * L + 1
    baseS = np.zeros((4 * M, 1), np.float32)
    for k in range(4 * M):
        baseS[k, 0] = (k // (2 * M)) * L + 1

    cf32 = np.zeros((128, CF32_COLS), np.float32)
    cf32[0:8, 0:96] = rep8
    cf32[0:96, 96:98] = baseA
    cf32[0:16, 98:99] = baseS
    cf32[0:128, IDF0:IDF0 + 128] = np.eye(128)
    # bbc filled per-call (bias input)

    cb16 = np.zeros((128, CB16_COLS), ml_dtypes.bfloat16)
    for y in range(BLK):
        for p in range(128):
            cb16[(p // BLK) * BLK + y, RYS0 + y * 128 + p] = 1.0
    cb16[0:16, SELE0:SELE0 + 4] = selE
    cb16[0:96, SELA0:SELA0 + 24] = selA
    cb16[0:12, W120:W120 + 1] = 1.0 / HEADS
    cb16[0:1, SELBH0:SELBH0 + 4] = np.array([1.0, 0.0, 1.0, 0.0])
    cb16[0:1, SELBH0 + 4:SELBH0 + 8] = np.array([0.0, 1.0, 0.0, 1.0])

    perm = np.empty(EMB * BLK, np.int64)
    for cch in range(NEMB):
        for y in range(BLK):
            for p in range(128):
                g = cch * 16 + p // BLK
                x = p % BLK
                perm[(cch * BLK + y) * 128 + p] = g * 64 + x * BLK + y
    return {"cf32": cf32, "cb16": cb16, "perm": perm}


def _build_program(stage=99):
    nc = bacc.Bacc("TRN2", target_bir_lowering=False, debug=False)

    seq_h = nc.dram_tensor("seq", [BPC * L, HID], BF16, kind="ExternalInput")
    attn_h = nc.dram_tensor("attn", [BPC * HEADS * L, L], BF16, kind="ExternalInput")
    pos_h = nc.dram_tensor("pos", [4 * M, 1], I32, kind="ExternalInput")
    posb_hs = [
        nc.dram_tensor(f"posb{b}", [2 * M, 1], I32, kind="ExternalInput")
        for b in range(BPC)
    ]
    ner_h = nc.dram_tensor("ner", [NER, 4], F32, kind="ExternalInput")
    whT_h = nc.dram_tensor("whT", [CAT, EMB], BF16, kind="ExternalInput")
    wtT_h = nc.dram_tensor("wtT", [CAT, EMB], BF16, kind="ExternalInput")
    wbT_h = nc.dram_tensor("wbT", [EMB * BLK, 128], BF16, kind="ExternalInput")
    cf32_h = nc.dram_tensor("cf32", [128, CF32_COLS], F32, kind="ExternalInput")
    cb16_h = nc.dram_tensor("cb16", [128, CB16_COLS], BF16, kind="ExternalInput")
    out_h = nc.dram_tensor("logitsT", [NCLS, BPC], F32, kind="ExternalOutput")

    with TileContext(nc) as tc:
        with (
            tc.tile_pool(name="const", bufs=1) as cp,
            tc.tile_pool(name="data", bufs=1) as dp,
            tc.tile_pool(name="psbig", bufs=1, space="PSUM") as psb,
            tc.tile_pool(name="psea", bufs=2, space="PSUM") as pse,
            tc.tile_pool(name="pssm", bufs=3, space="PSUM") as pss,
        ):
            # ---- bulk loads ----
            cf = cp.tile([128, CF32_COLS], F32)
            nc.sync.dma_start(cf[:], cf32_h[:])
            cb = cp.tile([128, CB16_COLS], BF16)
            nc.sync.dma_start(cb[:], cb16_h[:])
            rep8 = cf[0:8, 0:96]
            baseA = cf[0:96, 96:98]
            baseS = cf[0:16, 98:99]
            bbc = cf[0:97, 99:100]
            rys = cb[:, RYS0:RYS0 + 1024]
            selE = cb[0:16, SELE0:SELE0 + 4]
            selA = cb[0:96, SELA0:SELA0 + 24]
            w12 = cb[0:12, W120:W120 + 1]
            bhr = cb[0:1, BHR0:BHR0 + EMB]
            btr = cb[0:1, BTR0:BTR0 + EMB]
            selbh = cb[0:1, SELBH0:SELBH0 + 4]
            selbt = cb[0:1, SELBH0 + 4:SELBH0 + 8]
            idf = cf[:, IDF0:IDF0 + 128]

            whs = cp.tile([128, KCH * EMB], BF16)
            nc.sync.dma_start(
                whs[:].rearrange("p (j d) -> p j d", j=KCH),
                whT_h[0:KCH * 128, :].rearrange("(j p) d -> p j d", p=128))
            whn = cp.tile([NER, EMB], BF16)
            nc.sync.dma_start(whn[:], whT_h[KCH * 128:CAT, :])
            wts = cp.tile([128, KCH * EMB], BF16)
            nc.sync.dma_start(
                wts[:].rearrange("p (j d) -> p j d", j=KCH),
                wtT_h[0:KCH * 128, :].rearrange("(j p) d -> p j d", p=128))
            wtn = cp.tile([NER, EMB], BF16)
            nc.sync.dma_start(wtn[:], wtT_h[KCH * 128:CAT, :])
            wbs = cp.tile([128, NBL * 128], BF16)
            nc.sync.dma_start(
                wbs[:].rearrange("p (k m) -> p k m", k=NBL),
                wbT_h[:].rearrange("(k p) m -> p k m", p=128))

            seqt = []
            for b in range(BPC):
                t = dp.tile([128, NL * HID], BF16, tag=f"seq{b}")
                nc.sync.dma_start(
                    t[:].rearrange("p (c d) -> p c d", c=NL),
                    seq_h[b * L:(b + 1) * L, :].rearrange("(c p) d -> p c d", p=128))
                seqt.append(t)

            ner4f = dp.tile([NER, 4], F32)
            nc.sync.dma_start(ner4f[:], ner_h[:])
            ner4 = dp.tile([NER, 4], BF16)
            nc.vector.tensor_copy(ner4[:], ner4f[:])

            # ---- index computation ----
            posi = dp.tile([4 * M, 1], I32)
            nc.sync.dma_start(posi[:], pos_h[:])
            posf = dp.tile([4 * M, 1], F32)
            nc.vector.tensor_copy(posf[:], posi[:])
            idxsf = dp.tile([4 * M, 1], F32)
            nc.vector.tensor_add(idxsf[:], posf[:], baseS)
            idxs = dp.tile([4 * M, 1], I32)
            nc.vector.tensor_copy(idxs[:], idxsf[:])

            idxa = []
            for b in range(BPC):
                posbi = dp.tile([2 * M, 1], I32, tag=f"posbi{b}")
                nc.sync.dma_start(posbi[:], posb_hs[b][:])
                posbf = dp.tile([2 * M, 1], F32, tag=f"posbf{b}")
                nc.vector.tensor_copy(posbf[:], posbi[:])
                ps_idx = pss.tile([2 * M * HEADS, 1], F32, tag="sm")
                nc.tensor.matmul(ps_idx[:], lhsT=rep8, rhs=posbf[:],
                                 start=True, stop=True)
                idxaf = dp.tile([2 * M * HEADS, 1], F32, tag=f"idxaf{b}")
                nc.vector.tensor_add(idxaf[:], ps_idx[:], baseA[:, b:b + 1])
                ia = dp.tile([2 * M * HEADS, 1], I32, tag=f"idxa{b}")
                nc.vector.tensor_copy(ia[:], idxaf[:])
                idxa.append(ia)

            # ---- gathers ----
            sg = dp.tile([4 * M, HID], BF16)
            nc.gpsimd.indirect_dma_start(
                out=sg[:], out_offset=None, in_=seq_h[:],
                in_offset=bass.IndirectOffsetOnAxis(ap=idxs[:, :1], axis=0))
            at = []
            for b in range(BPC):
                t = dp.tile([2 * M * HEADS, L], BF16, tag=f"at{b}")
                nc.gpsimd.indirect_dma_start(
                    out=t[:], out_offset=None, in_=attn_h[:],
                    in_offset=bass.IndirectOffsetOnAxis(ap=idxa[b][:, :1], axis=0))
                at.append(t)

            if stage < 1:
                lg = dp.tile([NCLS, BPC], F32)
                nc.vector.memset(lg[:], 0.0)
                nc.vector.tensor_copy(lg[0:4 * M, 0:1], sg[:, 0:1])
                nc.vector.tensor_copy(lg[0:2 * M * HEADS - 31, 1:2], at[0][0:2 * M * HEADS - 31, 0:1])
                nc.sync.dma_start(out_h[:], lg[:])
                return _finish(nc)

            # ---- entity embeddings: log-sum-exp over mentions ----
            exps = dp.tile([4 * M, HID], BF16)
            nc.scalar.activation(exps[:], sg[:], AF.Exp)
            ps_e = psb.tile([4, HID], F32, tag="big")
            for n0, nl_ in ((0, 512), (512, 256)):
                nc.tensor.matmul(ps_e[:, n0:n0 + nl_], lhsT=selE,
                                 rhs=exps[:, n0:n0 + nl_], start=True, stop=True)
            ent = dp.tile([4, HID], F32)
            nc.scalar.activation(ent[:], ps_e[:], AF.Ln)
            ps_et = pss.tile([128, 4 * NEMB], F32, tag="sm")
            for c in range(NEMB):
                nc.tensor.transpose(ps_et[:, c * 4:(c + 1) * 4],
                                    ent[:, c * 128:(c + 1) * 128], idf[0:4, 0:4])
            entT = dp.tile([128, 4 * NEMB], BF16)
            nc.vector.tensor_copy(entT[:], ps_et[:])

            # ---- entity attention pooling + context vector ----
            htc = []
            for b in range(BPC):
                ps_eah = pse.tile([HEADS, L], F32, tag="ea")
                nc.tensor.matmul(ps_eah[:], lhsT=selA[:, 0:HEADS], rhs=at[b][:],
                                 start=True, stop=True)
                ps_eat = pse.tile([HEADS, L], F32, tag="ea")
                nc.tensor.matmul(ps_eat[:], lhsT=selA[:, HEADS:2 * HEADS],
                                 rhs=at[b][:], start=True, stop=True)
                eah = dp.tile([HEADS, L], F32, tag=f"eah{b}")
                nc.vector.tensor_copy(eah[:], ps_eah[:])
                prd = dp.tile([HEADS, L], BF16, tag=f"prd{b}")
                nc.vector.tensor_tensor(out=prd[:], in0=eah[:], in1=ps_eat[:],
                                        op=ALU.mult)
                ps_ht = pss.tile([1, L], F32, tag="sm")
                nc.tensor.matmul(ps_ht[:], lhsT=w12, rhs=prd[:],
                                 start=True, stop=True)
                sm = dp.tile([1, 1], F32, tag=f"sm{b}")
                nc.vector.reduce_sum(sm[:], ps_ht[:], axis=mybir.AxisListType.X)
                den = dp.tile([1, 1], F32, tag=f"den{b}")
                nc.vector.tensor_scalar_add(den[:], sm[:], 1e-5)
                rcp = dp.tile([1, 1], F32, tag=f"rcp{b}")
                nc.vector.reciprocal(rcp[:], den[:])
                htn = dp.tile([1, L], F32, tag=f"htn{b}")
                nc.vector.tensor_scalar_mul(htn[:], ps_ht[:], rcp[:, :1])
                ps_htc = pss.tile([128, NL], F32, tag="sm")
                for c in range(NL):
                    nc.tensor.transpose(ps_htc[:, c:c + 1],
                                        htn[:, c * 128:(c + 1) * 128],
                                        idf[0:1, 0:1])
                h = dp.tile([128, NL], BF16, tag=f"htc{b}")
                nc.vector.tensor_copy(h[:], ps_htc[:])
                htc.append(h)

            if stage < 2:
                lg = dp.tile([NCLS, BPC], F32)
                nc.vector.memset(lg[:], 0.0)
                nc.vector.tensor_copy(lg[0:97, 0:1], entT[0:97, 0:1])
                nc.vector.tensor_copy(lg[0:97, 1:2], htc[0][0:97, 0:1])
                nc.sync.dma_start(out_h[:], lg[:])
                return _finish(nc)

            # ---- rs = seq^T @ ht  (column form) ----
            ps_rsc = pss.tile([128, NEMB * BPC], F32, tag="sm")
            for b in range(BPC):
                for d in range(NEMB):
                    for c in range(NL):
                        nc.tensor.matmul(
                            ps_rsc[:, d * BPC + b:d * BPC + b + 1],
                            lhsT=seqt[b][:, c * HID + d * 128:c * HID + (d + 1) * 128],
                            rhs=htc[b][:, c:c + 1],
                            start=(c == 0), stop=(c == NL - 1))
            rsc = dp.tile([128, 4 * NEMB], BF16)
            nc.vector.tensor_copy(
                rsc[:].rearrange("p (r b m) -> p r b m", r=NEMB, b=BPC),
                ps_rsc[:].rearrange("p (r b) -> p r b", r=NEMB)
                .unsqueeze(3).broadcast_to([128, NEMB, BPC, 2]))

            # ---- extractor GEMVs:  [4,768] = cat4^T @ W^T  ----
            def cat_chunk(j):
                if j < NEMB:
                    return entT[:, j * 4:(j + 1) * 4]
                if j < 2 * NEMB:
                    return rsc[:, (j - NEMB) * 4:(j - NEMB + 1) * 4]
                return ner4[:]

            t4 = []
            for wi, (ws, wn, selb, br) in enumerate(
                    ((whs, whn, selbh, bhr), (wts, wtn, selbt, btr))):
                ps_w = psb.tile([4, EMB], F32, tag="big")
                for n0, nl_ in ((0, 512), (512, 256)):
                    for j in range(KCH + 1):
                        lhsT = cat_chunk(j)
                        rhs = (ws[:, j * EMB + n0:j * EMB + n0 + nl_] if j < KCH
                               else wn[:, n0:n0 + nl_])
                        nc.tensor.matmul(ps_w[:, n0:n0 + nl_], lhsT=lhsT, rhs=rhs,
                                         start=(j == 0), stop=False)
                    nc.tensor.matmul(ps_w[:, n0:n0 + nl_], lhsT=selb,
                                     rhs=br[:, n0:n0 + nl_], start=False, stop=True)
                t = dp.tile([4, EMB], F32, tag=f"t4_{wi}")
                nc.scalar.activation(t[:], ps_w[:], AF.Tanh)
                t4.append(t)

            if stage < 3:
                lg = dp.tile([NCLS, BPC], F32)
                nc.vector.memset(lg[:], 0.0)
                nc.vector.tensor_copy(lg[0:4, 0:2], t4[0][:, 0:2])
                nc.vector.tensor_copy(lg[0:4, 1:2], t4[1][:, 0:1])
                nc.vector.tensor_copy(lg[0:89, 0:1], rsc[0:89, 0:1])
                nc.sync.dma_start(out_h[:], lg[:])
                return _finish(nc)

            # ---- transpose hs2/ts2 to columns ----
            ps_a = pss.tile([128, 4 * NEMB], F32, tag="sm")
            ps_b2 = pss.tile([128, 4 * NEMB], F32, tag="sm")
            for c in range(NEMB):
                nc.tensor.transpose(ps_a[:, c * 4:(c + 1) * 4],
                                    t4[0][:, c * 128:(c + 1) * 128], idf[0:4, 0:4])
                nc.tensor.transpose(ps_b2[:, c * 4:(c + 1) * 4],
                                    t4[1][:, c * 128:(c + 1) * 128], idf[0:4, 0:4])
            h2t = dp.tile([128, 4 * NEMB], BF16)
            nc.vector.tensor_copy(
                h2t[:].rearrange("p (c b) -> p c b", c=NEMB)[:, :, 0:4:2],
                ps_a[:].rearrange("p (c b) -> p c b", c=NEMB)[:, :, 0:4:2])
            nc.vector.tensor_copy(
                h2t[:].rearrange("p (c b) -> p c b", c=NEMB)[:, :, 1:4:2],
                ps_b2[:].rearrange("p (c b) -> p c b", c=NEMB)[:, :, 1:4:2])

            if stage < 4:
                lg = dp.tile([NCLS, BPC], F32)
                nc.vector.memset(lg[:], 0.0)
                nc.vector.tensor_copy(lg[0:97, 0:2], h2t[0:97, 0:2])
                nc.sync.dma_start(out_h[:], lg[:])
                return _finish(nc)

            # ---- grouped bilinear + classifier ----
            # ts-replication: out col layout (y, c, b) = y*12 + c*2 + b
            ps_t2x = pss.tile([128, BLK * NEMB * BPC], F32, tag="sm")
            tscols = h2t[:].rearrange("p (c b) -> p c b", c=NEMB)[:, :, 1:4:2]
            for y in range(BLK):
                nc.tensor.matmul(
                    ps_t2x[:, y * 12:(y + 1) * 12]
                    .rearrange("p (c b) -> p c b", c=NEMB),
                    lhsT=rys[:, y * 128:(y + 1) * 128],
                    rhs=tscols, start=True, stop=True)
            if stage < 5:
                lg = dp.tile([NCLS, BPC], F32)
                nc.vector.memset(lg[:], 0.0)
                nc.vector.tensor_copy(lg[0:97, 0:2], ps_t2x[0:97, 0:2])
                nc.sync.dma_start(out_h[:], lg[:])
                return _finish(nc)

            blt = dp.tile([128, NEMB * 16], BF16)
            for c in range(NEMB):
                nc.vector.tensor_tensor(
                    out=blt[:, c * 16:(c + 1) * 16]
                    .rearrange("p (y b) -> p y b", y=BLK),
                    in0=h2t[:, c * 4:c * 4 + 4:2].unsqueeze(1)
                        .broadcast_to([128, BLK, 2]),
                    in1=ps_t2x[:].rearrange("p (y c b) -> p y c b", y=BLK, c=NEMB)
                    [:, :, c, :],
                    op=ALU.mult)
            if stage < 6:
                lg = dp.tile([NCLS, BPC], F32)
                nc.vector.memset(lg[:], 0.0)
                nc.vector.tensor_copy(lg[0:97, 0:2], blt[0:97, 0:2])
                nc.sync.dma_start(out_h[:], lg[:])
                return _finish(nc)

            ps_l = pss.tile([NCLS, BPC], F32, tag="sm")
            for c in range(NEMB):
                for y in range(BLK):
                    k = c * BLK + y
                    nc.tensor.matmul(ps_l[:], lhsT=wbs[:, k * 128:k * 128 + NCLS],
                                     rhs=blt[:, c * 16 + y * 2:c * 16 + y * 2 + 2],
                                     start=(k == 0), stop=(k == NBL - 1))
            lg = dp.tile([NCLS, BPC], F32)
            if stage < 7:
                nc.vector.memset(lg[:], 0.0)
                nc.vector.tensor_copy(lg[0:1, 0:1], ps_l[0:1, 0:1])
            else:
                nc.vector.tensor_scalar_add(lg[:], ps_l[:], bbc[:, :1])
            nc.sync.dma_start(out_h[:], lg[:])

    return _finish(nc)


def _finish(nc):
    return nc


def _get_program():
    if "nc" not in _cache:
        nc = _build_program()
        nc.finalize()
        _cache["nc"] = nc
        _cache["consts"] = _build_constants()
    return _cache["nc"], _cache["consts"]


def kernel(sequence_output, attention, entity_pos, hs_ner_tags, ts_ner_tags,
           Wh, bh, Wt, bt, Wb, bb):
    nc, c = _get_program()

    seq = np.asarray(sequence_output, dtype=np.float32).astype(ml_dtypes.bfloat16)
    attn = np.asarray(attention, dtype=np.float32).astype(ml_dtypes.bfloat16)
    pos = np.asarray(entity_pos).astype(np.int32)
    nh = np.asarray(hs_ner_tags, dtype=np.float32)
    nt = np.asarray(ts_ner_tags, dtype=np.float32)
    whT = np.ascontiguousarray(np.asarray(Wh, dtype=np.float32).T).astype(ml_dtypes.bfloat16)
    wtT = np.ascontiguousarray(np.asarray(Wt, dtype=np.float32).T).astype(ml_dtypes.bfloat16)
    wbT = np.ascontiguousarray(np.asarray(Wb, dtype=np.float32).T)[c["perm"]]
    wbT = np.pad(wbT, ((0, 0), (0, 128 - NCLS))).astype(ml_dtypes.bfloat16)

    cb16 = c["cb16"].copy()
    cb16[0:1, BHR0:BHR0 + EMB] = np.asarray(bh, np.float32).reshape(1, EMB)
    cb16[32:33, BHR0:BHR0 + EMB] = np.asarray(bt, np.float32).reshape(1, EMB)
    cf32 = c["cf32"].copy()
    cf32[0:97, 99] = np.asarray(bb, np.float32)

    in_maps = []
    for core in range(NCORES):
        b0 = core * BPC
        pc = np.ascontiguousarray(pos[b0:b0 + BPC])          # [2,2,M]
        ner = np.stack([nh[b0], nt[b0], nh[b0 + 1], nt[b0 + 1]], axis=1)
        im = {
            "seq": np.ascontiguousarray(seq[b0:b0 + BPC]).reshape(BPC * L, HID),
            "attn": np.ascontiguousarray(attn[b0:b0 + BPC]).reshape(BPC * HEADS * L, L),
            "pos": pc.reshape(4 * M, 1),
            "ner": np.ascontiguousarray(ner.astype(np.float32)),
            "whT": whT, "wtT": wtT, "wbT": wbT,
            "cf32": cf32, "cb16": cb16,
        }
        for b in range(BPC):
            im[f"posb{b}"] = np.ascontiguousarray(pc[b].reshape(2 * M, 1))
        in_maps.append(im)

    res = run_bass_kernel_spmd(nc, in_maps, core_ids=list(range(NCORES)))
    _cache["last_res"] = res
    out = np.empty((B, NCLS), np.float32)
    for core in range(NCORES):
        out[core * BPC:(core + 1) * BPC] = res.results[core]["logitsT"].T
    return out


# revision 16
# speedup vs baseline: 1.9294x; 1.1642x over previous
"""Trainium2 Bass kernel for BertWithAdaThresholdLocContextPooling.

Strategy: pure data parallel over batch (B=16 -> 2 batches per core x 8 cores).
Each core:
  - gathers mention rows of sequence_output / attention via indirect DMA
    (only ~0.2MB of the 12.6MB attention shard is ever read from HBM),
  - logsumexp-pools mention embeddings, mean-pools attention rows,
  - computes the localized-context attention rs = seq^T @ ht,
  - runs the two extractor GEMVs (bf16 data, fp32 accumulate),
  - forms the grouped bilinear via PE replication matmuls,
  - applies the classifier Wb.
Weights are replicated to all cores; the host pre-transposes/casts them and
packs small constants so each core issues only a handful of large DMAs.
"""

import sys

for _p in ("/opt/trn_rl_repo",):
    if _p not in sys.path:
        sys.path.insert(0, _p)

import numpy as np
import ml_dtypes

import concourse.bacc as bacc
import concourse.bass as bass
import concourse.mybir as mybir
from concourse.tile import TileContext
from concourse.bass_utils import run_bass_kernel_spmd

F32 = mybir.dt.float32
BF16 = mybir.dt.bfloat16
I32 = mybir.dt.int32
AF = mybir.ActivationFunctionType
ALU = mybir.AluOpType

B, L, HID = 16, 512, 768
HEADS, M = 12, 4
EMB, BLK, NER, NCLS = 768, 8, 6, 97
NCORES = 8
BPC = B // NCORES          # batches per core = 2
CAT = 2 * HID + NER        # 1542
KCH = 12                   # full 128-row contraction chunks of CAT
NEMB = EMB // 128          # 6 chunks of EMB
NL = L // 128              # 4 chunks of L
NBL = EMB * BLK // 128     # 48 classifier contraction chunks

# packed-constant layouts
# CF32 [128, 228]: rep8 [0:8,0:96] | baseA [0:96,96:98] | baseS [0:16,98:99]
#                  | bbc [0:97,99:100] | identity f32 [0:128,100:228]
IDF0 = 100
CF32_COLS = 228
# CB16 [128, 1924]: rys [0:128,0:1024] | selE [0:16,1024:1028]
#   | selA [0:96,1028:1052] | w12 [0:12,1052:1053] | bhr [0:1,1053:1821->no]
# (bhr/btr need legal matmul row bases: bhr row 0, btr row 32)
RYS0 = 0
SELE0 = 1024
SELA0 = 1028
W120 = 1052
BHR0 = 1056
BTR0 = BHR0 + 768
SELBH0 = BTR0 + 768
CB16_COLS = SELBH0 + 8

_cache = {}


def _build_constants():
    selE = np.zeros((4 * M, 4), np.float32)
    for k in range(4 * M):
        selE[k, k // M] = 1.0
    selA = np.zeros((2 * M * HEADS, 2 * HEADS), np.float32)
    for i in range(2):
        for m in range(M):
            for h in range(HEADS):
                selA[i * M * HEADS + m * HEADS + h, i * HEADS + h] = 1.0 / M
    rep8 = np.zeros((2 * M, 2 * M * HEADS), np.float32)
    for q in range(2 * M * HEADS):
        rep8[q // HEADS, q] = 1.0
    baseA = np.zeros((2 * M * HEADS, BPC), np.float32)
    for q in range(2 * M * HEADS):
        for b in range(BPC):
            baseA[q, b] = (b * HEADS + q % HEADS) * L + 1
    baseS = np.zeros((4 * M, 1), np.float32)
    for k in range(4 * M):
        baseS[k, 0] = (k // (2 * M)) * L + 1

    cf32 = np.zeros((128, CF32_COLS), np.float32)
    cf32[0:8, 0:96] = rep8
    cf32[0:96, 96:98] = baseA
    cf32[0:16, 98:99] = baseS
    cf32[0:128, IDF0:IDF0 + 128] = np.eye(128)
    # bbc filled per-call (bias input)

    cb16 = np.zeros((128, CB16_COLS), ml_dtypes.bfloat16)
    for y in range(BLK):
        for p in range(128):
            cb16[(p // BLK) * BLK + y, RYS0 + y * 128 + p] = 1.0
    cb16[0:16, SELE0:SELE0 + 4] = selE
    cb16[0:96, SELA0:SELA0 + 24] = selA
    cb16[0:12, W120:W120 + 1] = 1.0 / HEADS
    cb16[0:1, SELBH0:SELBH0 + 4] = np.array([1.0, 0.0, 1.0, 0.0])
    cb16[0:1, SELBH0 + 4:SELBH0 + 8] = np.array([0.0, 1.0, 0.0, 1.0])

    perm = np.empty(EMB * BLK, np.int64)
    for cch in range(NEMB):
        for y in range(BLK):
            for p in range(128):
                g = cch * 16 + p // BLK
                x = p % BLK
                perm[(cch * BLK + y) * 128 + p] = g * 64 + x * BLK + y
    return {"cf32": cf32, "cb16": cb16, "perm": perm}


def _build_program(stage=99):
    nc = bacc.Bacc("TRN2", target_bir_lowering=False, debug=False)

    seq_h = nc.dram_tensor("seq", [BPC * L, HID], BF16, kind="ExternalInput")
    attn_h = nc.dram_tensor("attn", [BPC * HEADS * L, L], BF16, kind="ExternalInput")
    pos_h = nc.dram_tensor("pos", [4 * M, 1], I32, kind="ExternalInput")
    posb_hs = [
        nc.dram_tensor(f"posb{b}", [2 * M, 1], I32, kind="ExternalInput")
        for b in range(BPC)
    ]
    ner_h = nc.dram_tensor("ner", [NER, 4], F32, kind="ExternalInput")
    whs_h = nc.dram_tensor("whs", [128, KCH * EMB + EMB], BF16, kind="ExternalInput")
    wts_h = nc.dram_tensor("wts", [128, KCH * EMB + EMB], BF16, kind="ExternalInput")
    wbs_h = nc.dram_tensor("wbs", [128, NBL * 128], BF16, kind="ExternalInput")
    cf32_h = nc.dram_tensor("cf32", [128, CF32_COLS], F32, kind="ExternalInput")
    cb16_h = nc.dram_tensor("cb16", [128, CB16_COLS], BF16, kind="ExternalInput")
    out_h = nc.dram_tensor("logitsT", [NCLS, BPC], F32, kind="ExternalOutput")

    with TileContext(nc) as tc:
        with (
            tc.tile_pool(name="const", bufs=1) as cp,
            tc.tile_pool(name="data", bufs=1) as dp,
            tc.tile_pool(name="psbig", bufs=1, space="PSUM") as psb,
            tc.tile_pool(name="psea", bufs=2, space="PSUM") as pse,
            tc.tile_pool(name="pssm", bufs=3, space="PSUM") as pss,
        ):
            # ---- critical small loads first (sync queue) ----
            posi = dp.tile([4 * M, 1], I32)
            nc.sync.dma_start(posi[:], pos_h[:])
            posbi = []
            for b in range(BPC):
                t = dp.tile([2 * M, 1], I32, tag=f"posbi{b}")
                nc.sync.dma_start(t[:], posb_hs[b][:])
                posbi.append(t)
            cf = cp.tile([128, CF32_COLS], F32)
            nc.sync.dma_start(cf[:], cf32_h[:])
            cb = cp.tile([128, CB16_COLS], BF16)
            nc.sync.dma_start(cb[:], cb16_h[:])
            rep8 = cf[0:8, 0:96]
            baseA = cf[0:96, 96:98]
            baseS = cf[0:16, 98:99]
            bbc = cf[0:97, 99:100]
            rys = cb[:, RYS0:RYS0 + 1024]
            selE = cb[0:16, SELE0:SELE0 + 4]
            selA = cb[0:96, SELA0:SELA0 + 24]
            w12 = cb[0:12, W120:W120 + 1]
            bhr = cb[0:1, BHR0:BHR0 + EMB]
            btr = cb[0:1, BTR0:BTR0 + EMB]
            selbh = cb[0:1, SELBH0:SELBH0 + 4]
            selbt = cb[0:1, SELBH0 + 4:SELBH0 + 8]
            idf = cf[:, IDF0:IDF0 + 128]

            ner4f = dp.tile([NER, 4], F32)
            nc.sync.dma_start(ner4f[:], ner_h[:])
            ner4 = dp.tile([NER, 4], BF16)
            nc.vector.tensor_copy(ner4[:], ner4f[:])

            seqt = []
            for b in range(BPC):
                t = dp.tile([128, NL * HID], BF16, tag=f"seq{b}")
                nc.sync.dma_start(
                    t[:].rearrange("p (c d) -> p c d", c=NL),
                    seq_h[b * L:(b + 1) * L, :].rearrange("(c p) d -> p c d", p=128))
                seqt.append(t)

            # ---- bulk weight loads (scalar queue; host pre-rearranged) ----
            whsf = cp.tile([128, KCH * EMB + EMB], BF16)
            nc.scalar.dma_start(whsf[:], whs_h[:])
            whs = whsf[:, 0:KCH * EMB]
            whn = whsf[0:NER, KCH * EMB:KCH * EMB + EMB]
            wtsf = cp.tile([128, KCH * EMB + EMB], BF16)
            nc.scalar.dma_start(wtsf[:], wts_h[:])
            wts = wtsf[:, 0:KCH * EMB]
            wtn = wtsf[0:NER, KCH * EMB:KCH * EMB + EMB]
            wbs = cp.tile([128, NBL * 128], BF16)
            nc.scalar.dma_start(wbs[:], wbs_h[:])

            # ---- index computation ----
            posf = dp.tile([4 * M, 1], F32)
            nc.vector.tensor_copy(posf[:], posi[:])
            idxsf = dp.tile([4 * M, 1], F32)
            nc.vector.tensor_add(idxsf[:], posf[:], baseS)
            idxs = dp.tile([4 * M, 1], I32)
            nc.vector.tensor_copy(idxs[:], idxsf[:])

            idxa = []
            for b in range(BPC):
                posbf = dp.tile([2 * M, 1], F32, tag=f"posbf{b}")
                nc.vector.tensor_copy(posbf[:], posbi[b][:])
                ps_idx = pss.tile([2 * M * HEADS, 1], F32, tag="sm")
                nc.tensor.matmul(ps_idx[:], lhsT=rep8, rhs=posbf[:],
                                 start=True, stop=True)
                idxaf = dp.tile([2 * M * HEADS, 1], F32, tag=f"idxaf{b}")
                nc.vector.tensor_add(idxaf[:], ps_idx[:], baseA[:, b:b + 1])
                ia = dp.tile([2 * M * HEADS, 1], I32, tag=f"idxa{b}")
                nc.vector.tensor_copy(ia[:], idxaf[:])
                idxa.append(ia)

            # ---- gathers ----
            sg = dp.tile([4 * M, HID], BF16)
            nc.gpsimd.indirect_dma_start(
                out=sg[:], out_offset=None, in_=seq_h[:],
                in_offset=bass.IndirectOffsetOnAxis(ap=idxs[:, :1], axis=0))
            at = []
            for b in range(BPC):
                t = dp.tile([2 * M * HEADS, L], BF16, tag=f"at{b}")
                nc.gpsimd.indirect_dma_start(
                    out=t[:], out_offset=None, in_=attn_h[:],
                    in_offset=bass.IndirectOffsetOnAxis(ap=idxa[b][:, :1], axis=0))
                at.append(t)

            if stage < 1:
                lg = dp.tile([NCLS, BPC], F32)
                nc.vector.memset(lg[:], 0.0)
                nc.vector.tensor_copy(lg[0:4 * M, 0:1], sg[:, 0:1])
                nc.vector.tensor_copy(lg[0:2 * M * HEADS - 31, 1:2], at[0][0:2 * M * HEADS - 31, 0:1])
                nc.sync.dma_start(out_h[:], lg[:])
                return _finish(nc)

            # ---- entity embeddings: log-sum-exp over mentions ----
            exps = dp.tile([4 * M, HID], BF16)
            nc.scalar.activation(exps[:], sg[:], AF.Exp)
            ps_e = psb.tile([4, HID], F32, tag="big")
            for n0, nl_ in ((0, 512), (512, 256)):
                nc.tensor.matmul(ps_e[:, n0:n0 + nl_], lhsT=selE,
                                 rhs=exps[:, n0:n0 + nl_], start=True, stop=True)
            ent = dp.tile([4, HID], F32)
            nc.scalar.activation(ent[:], ps_e[:], AF.Ln)
            ps_et = pss.tile([128, 4 * NEMB], F32, tag="sm")
            for c in range(NEMB):
                nc.tensor.transpose(ps_et[:, c * 4:(c + 1) * 4],
                                    ent[:, c * 128:(c + 1) * 128], idf[0:4, 0:4])
            entT = dp.tile([128, 4 * NEMB], BF16)
            nc.vector.tensor_copy(entT[:], ps_et[:])

            # ---- entity attention pooling + context vector ----
            htc = []
            for b in range(BPC):
                ps_eah = pse.tile([HEADS, L], F32, tag="ea")
                nc.tensor.matmul(ps_eah[:], lhsT=selA[:, 0:HEADS], rhs=at[b][:],
                                 start=True, stop=True)
                ps_eat = pse.tile([HEADS, L], F32, tag="ea")
                nc.tensor.matmul(ps_eat[:], lhsT=selA[:, HEADS:2 * HEADS],
                                 rhs=at[b][:], start=True, stop=True)
                eah = dp.tile([HEADS, L], F32, tag=f"eah{b}")
                nc.vector.tensor_copy(eah[:], ps_eah[:])
                prd = dp.tile([HEADS, L], BF16, tag=f"prd{b}")
                nc.vector.tensor_tensor(out=prd[:], in0=eah[:], in1=ps_eat[:],
                                        op=ALU.mult)
                ps_ht = pss.tile([1, L], F32, tag="sm")
                nc.tensor.matmul(ps_ht[:], lhsT=w12, rhs=prd[:],
                                 start=True, stop=True)
                sm = dp.tile([1, 1], F32, tag=f"sm{b}")
                nc.vector.reduce_sum(sm[:], ps_ht[:], axis=mybir.AxisListType.X)
                den = dp.tile([1, 1], F32, tag=f"den{b}")
                nc.vector.tensor_scalar_add(den[:], sm[:], 1e-5)
                rcp = dp.tile([1, 1], F32, tag=f"rcp{b}")
                nc.vector.reciprocal(rcp[:], den[:])
                htn = dp.tile([1, L], F32, tag=f"htn{b}")
                nc.vector.tensor_scalar_mul(htn[:], ps_ht[:], rcp[:, :1])
                ps_htc = pss.tile([128, NL], F32, tag="sm")
                for c in range(NL):
                    nc.tensor.transpose(ps_htc[:, c:c + 1],
                                        htn[:, c * 128:(c + 1) * 128],
                                        idf[0:1, 0:1])
                h = dp.tile([128, NL], BF16, tag=f"htc{b}")
                nc.vector.tensor_copy(h[:], ps_htc[:])
                htc.append(h)

            if stage < 2:
                lg = dp.tile([NCLS, BPC], F32)
                nc.vector.memset(lg[:], 0.0)
                nc.vector.tensor_copy(lg[0:97, 0:1], entT[0:97, 0:1])
                nc.vector.tensor_copy(lg[0:97, 1:2], htc[0][0:97, 0:1])
                nc.sync.dma_start(out_h[:], lg[:])
                return _finish(nc)

            # ---- rs = seq^T @ ht  (column form) ----
            ps_rsc = pss.tile([128, NEMB * BPC], F32, tag="sm")
            for b in range(BPC):
                for d in range(NEMB):
                    for c in range(NL):
                        nc.tensor.matmul(
                            ps_rsc[:, d * BPC + b:d * BPC + b + 1],
                            lhsT=seqt[b][:, c * HID + d * 128:c * HID + (d + 1) * 128],
                            rhs=htc[b][:, c:c + 1],
                            start=(c == 0), stop=(c == NL - 1))
            rsc = dp.tile([128, 4 * NEMB], BF16)
            nc.vector.tensor_copy(
                rsc[:].rearrange("p (r b m) -> p r b m", r=NEMB, b=BPC),
                ps_rsc[:].rearrange("p (r b) -> p r b", r=NEMB)
                .unsqueeze(3).broadcast_to([128, NEMB, BPC, 2]))

            # ---- extractor GEMVs:  [4,768] = cat4^T @ W^T  ----
            def cat_chunk(j):
                if j < NEMB:
                    return entT[:, j * 4:(j + 1) * 4]
                if j < 2 * NEMB:
                    return rsc[:, (j - NEMB) * 4:(j - NEMB + 1) * 4]
                return ner4[:]

            t4 = []
            for wi, (ws, wn, selb, br) in enumerate(
                    ((whs, whn, selbh, bhr), (wts, wtn, selbt, btr))):
                ps_w = psb.tile([4, EMB], F32, tag="big")
                for n0, nl_ in ((0, 512), (512, 256)):
                    for j in range(KCH + 1):
                        lhsT = cat_chunk(j)
                        rhs = (ws[:, j * EMB + n0:j * EMB + n0 + nl_] if j < KCH
                               else wn[:, n0:n0 + nl_])
                        nc.tensor.matmul(ps_w[:, n0:n0 + nl_], lhsT=lhsT, rhs=rhs,
                                         start=(j == 0), stop=False)
                    nc.tensor.matmul(ps_w[:, n0:n0 + nl_], lhsT=selb,
                                     rhs=br[:, n0:n0 + nl_], start=False, stop=True)
                t = dp.tile([4, EMB], F32, tag=f"t4_{wi}")
                nc.scalar.activation(t[:], ps_w[:], AF.Tanh)
                t4.append(t)

            if stage < 3:
                lg = dp.tile([NCLS, BPC], F32)
                nc.vector.memset(lg[:], 0.0)
                nc.vector.tensor_copy(lg[0:4, 0:2], t4[0][:, 0:2])
                nc.vector.tensor_copy(lg[0:4, 1:2], t4[1][:, 0:1])
                nc.vector.tensor_copy(lg[0:89, 0:1], rsc[0:89, 0:1])
                nc.sync.dma_start(out_h[:], lg[:])
                return _finish(nc)

            # ---- transpose hs2/ts2 to columns ----
            ps_a = pss.tile([128, 4 * NEMB], F32, tag="sm")
            ps_b2 = pss.tile([128, 4 * NEMB], F32, tag="sm")
            for c in range(NEMB):
                nc.tensor.transpose(ps_a[:, c * 4:(c + 1) * 4],
                                    t4[0][:, c * 128:(c + 1) * 128], idf[0:4, 0:4])
                nc.tensor.transpose(ps_b2[:, c * 4:(c + 1) * 4],
                                    t4[1][:, c * 128:(c + 1) * 128], idf[0:4, 0:4])
            h2t = dp.tile([128, 4 * NEMB], BF16)
            nc.vector.tensor_copy(
                h2t[:].rearrange("p (c b) -> p c b", c=NEMB)[:, :, 0:4:2],
                ps_a[:].rearrange("p (c b) -> p c b", c=NEMB)[:, :, 0:4:2])
            nc.vector.tensor_copy(
                h2t[:].rearrange("p (c b) -> p c b", c=NEMB)[:, :, 1:4:2],
                ps_b2[:].rearrange("p (c b) -> p c b", c=NEMB)[:, :, 1:4:2])

            if stage < 4:
                lg = dp.tile([NCLS, BPC], F32)
                nc.vector.memset(lg[:], 0.0)
                nc.vector.tensor_copy(lg[0:97, 0:2], h2t[0:97, 0:2])
                nc.sync.dma_start(out_h[:], lg[:])
                return _finish(nc)

            # ---- grouped bilinear + classifier ----
            # ts-replication: out col layout (y, c, b) = y*12 + c*2 + b
            ps_t2x = pss.tile([128, BLK * NEMB * BPC], F32, tag="sm")
            tscols = h2t[:].rearrange("p (c b) -> p c b", c=NEMB)[:, :, 1:4:2]
            for y in range(BLK):
                nc.tensor.matmul(
                    ps_t2x[:, y * 12:(y + 1) * 12]
                    .rearrange("p (c b) -> p c b", c=NEMB),
                    lhsT=rys[:, y * 128:(y + 1) * 128],
                    rhs=tscols, start=True, stop=True)
            if stage < 5:
                lg = dp.tile([NCLS, BPC], F32)
                nc.vector.memset(lg[:], 0.0)
                nc.vector.tensor_copy(lg[0:97, 0:2], ps_t2x[0:97, 0:2])
                nc.sync.dma_start(out_h[:], lg[:])
                return _finish(nc)

            blt = dp.tile([128, NEMB * 16], BF16)
            for c in range(NEMB):
                nc.vector.tensor_tensor(
                    out=blt[:, c * 16:(c + 1) * 16]
                    .rearrange("p (y b) -> p y b", y=BLK),
                    in0=h2t[:, c * 4:c * 4 + 4:2].unsqueeze(1)
                        .broadcast_to([128, BLK, 2]),
                    in1=ps_t2x[:].rearrange("p (y c b) -> p y c b", y=BLK, c=NEMB)
                    [:, :, c, :],
                    op=ALU.mult)
            if stage < 6:
                lg = dp.tile([NCLS, BPC], F32)
                nc.vector.memset(lg[:], 0.0)
                nc.vector.tensor_copy(lg[0:97, 0:2], blt[0:97, 0:2])
                nc.sync.dma_start(out_h[:], lg[:])
                return _finish(nc)

            ps_l = pss.tile([NCLS, BPC], F32, tag="sm")
            for c in range(NEMB):
                for y in range(BLK):
                    k = c * BLK + y
                    nc.tensor.matmul(ps_l[:], lhsT=wbs[:, k * 128:k * 128 + NCLS],
                                     rhs=blt[:, c * 16 + y * 2:c * 16 + y * 2 + 2],
                                     start=(k == 0), stop=(k == NBL - 1))
            lg = dp.tile([NCLS, BPC], F32)
            if stage < 7:
                nc.vector.memset(lg[:], 0.0)
                nc.vector.tensor_copy(lg[0:1, 0:1], ps_l[0:1, 0:1])
            else:
                nc.vector.tensor_scalar_add(lg[:], ps_l[:], bbc[:, :1])
            nc.sync.dma_start(out_h[:], lg[:])

    return _finish(nc)


def _finish(nc):
    return nc


def _get_program():
    if "nc" not in _cache:
        nc = _build_program()
        nc.finalize()
        _cache["nc"] = nc
        _cache["consts"] = _build_constants()
    return _cache["nc"], _cache["consts"]


def kernel(sequence_output, attention, entity_pos, hs_ner_tags, ts_ner_tags,
           Wh, bh, Wt, bt, Wb, bb):
    nc, c = _get_program()

    seq = np.asarray(sequence_output, dtype=np.float32).astype(ml_dtypes.bfloat16)
    attn = np.asarray(attention, dtype=np.float32).astype(ml_dtypes.bfloat16)
    pos = np.asarray(entity_pos).astype(np.int32)
    nh = np.asarray(hs_ner_tags, dtype=np.float32)
    nt = np.asarray(ts_ner_tags, dtype=np.float32)
    whT = np.ascontiguousarray(np.asarray(Wh, dtype=np.float32).T).astype(ml_dtypes.bfloat16)
    wtT = np.ascontiguousarray(np.asarray(Wt, dtype=np.float32).T).astype(ml_dtypes.bfloat16)
    wbT = np.ascontiguousarray(np.asarray(Wb, dtype=np.float32).T)[c["perm"]]
    wbT = np.pad(wbT, ((0, 0), (0, 128 - NCLS))).astype(ml_dtypes.bfloat16)

    def sbuf_image(w, extra):
        main = w[0:KCH * 128].reshape(KCH, 128, EMB).transpose(1, 0, 2).reshape(128, KCH * EMB)
        img = np.zeros((128, KCH * EMB + EMB), ml_dtypes.bfloat16)
        img[:, 0:KCH * EMB] = main
        img[0:NER, KCH * EMB:] = extra
        return img

    whs = sbuf_image(whT, whT[KCH * 128:CAT])
    wts = sbuf_image(wtT, wtT[KCH * 128:CAT])
    wbs = wbT.reshape(NBL, 128, 128).transpose(1, 0, 2).reshape(128, NBL * 128)
    wbs = np.ascontiguousarray(wbs)

    cb16 = c["cb16"].copy()
    cb16[0:1, BHR0:BHR0 + EMB] = np.asarray(bh, np.float32).reshape(1, EMB)
    cb16[32:33, BHR0:BHR0 + EMB] = np.asarray(bt, np.float32).reshape(1, EMB)
    cf32 = c["cf32"].copy()
    cf32[0:97, 99] = np.asarray(bb, np.float32)

    in_maps = []
    for core in range(NCORES):
        b0 = core * BPC
        pc = np.ascontiguousarray(pos[b0:b0 + BPC])          # [2,2,M]
        ner = np.stack([nh[b0], nt[b0], nh[b0 + 1], nt[b0 + 1]], axis=1)
        im = {
            "seq": np.ascontiguousarray(seq[b0:b0 + BPC]).reshape(BPC * L, HID),
            "attn": np.ascontiguousarray(attn[b0:b0 + BPC]).reshape(BPC * HEADS * L, L),
            "pos": pc.reshape(4 * M, 1),
            "ner": np.ascontiguousarray(ner.astype(np.float32)),
            "whs": whs, "wts": wts, "wbs": wbs,
            "cf32": cf32, "cb16": cb16,
        }
        for b in range(BPC):
            im[f"posb{b}"] = np.ascontiguousarray(pc[b].reshape(2 * M, 1))
        in_maps.append(im)

    res = run_bass_kernel_spmd(nc, in_maps, core_ids=list(range(NCORES)))
    _cache["last_res"] = res
    out = np.empty((B, NCLS), np.float32)
    for core in range(NCORES):
        out[core * BPC:(core + 1) * BPC] = res.results[core]["logitsT"].T
    return out


# revision 17
# speedup vs baseline: 2.1247x; 1.1012x over previous
"""Trainium2 Bass kernel for BertWithAdaThresholdLocContextPooling.

Strategy: pure data parallel over batch (B=16 -> 2 batches per core x 8 cores).
Each core:
  - gathers mention rows of sequence_output / attention via indirect DMA
    (only ~0.2MB of the 12.6MB attention shard is ever read from HBM),
  - logsumexp-pools mention embeddings, mean-pools attention rows,
  - computes the localized-context attention rs = seq^T @ ht,
  - runs the two extractor GEMVs (bf16 data, fp32 accumulate),
  - forms the grouped bilinear via PE replication matmuls,
  - applies the classifier Wb.
Weights are replicated to all cores; the host pre-transposes/casts them and
packs small constants so each core issues only a handful of large DMAs.
"""

import sys

for _p in ("/opt/trn_rl_repo",):
    if _p not in sys.path:
        sys.path.insert(0, _p)

import numpy as np
import ml_dtypes

import concourse.bacc as bacc
import concourse.bass as bass
import concourse.mybir as mybir
from concourse.tile import TileContext
from concourse.bass_utils import run_bass_kernel_spmd

F32 = mybir.dt.float32
BF16 = mybir.dt.bfloat16
I32 = mybir.dt.int32
AF = mybir.ActivationFunctionType
ALU = mybir.AluOpType

B, L, HID = 16, 512, 768
HEADS, M = 12, 4
EMB, BLK, NER, NCLS = 768, 8, 6, 97
NCORES = 8
BPC = B // NCORES          # batches per core = 2
CAT = 2 * HID + NER        # 1542
KCH = 12                   # full 128-row contraction chunks of CAT
NEMB = EMB // 128          # 6 chunks of EMB
NL = L // 128              # 4 chunks of L
NBL = EMB * BLK // 128     # 48 classifier contraction chunks

# packed-constant layouts
# CIDX [96, 99] f32 (critical path): rep8 [0:8,0:96] | baseA [0:96,96:98]
#                                    | baseS [0:16,98:99]
CIDX_COLS = 99
# CID2 [128, 129] f32: identity [0:128,0:128] | bbc [0:97,128:129]
CID2_COLS = 129
# CB16 [128, 1924]: rys [0:128,0:1024] | selE [0:16,1024:1028]
#   | selA [0:96,1028:1052] | w12 [0:12,1052:1053] | bhr [0:1,1053:1821->no]
# (bhr/btr need legal matmul row bases: bhr row 0, btr row 32)
RYS0 = 0
SELE0 = 1024
SELA0 = 1028
W120 = 1052
BHR0 = 1056
BTR0 = BHR0 + 768
SELBH0 = BTR0 + 768
CB16_COLS = SELBH0 + 8

_cache = {}


def _build_constants():
    selE = np.zeros((4 * M, 4), np.float32)
    for k in range(4 * M):
        selE[k, k // M] = 1.0
    selA = np.zeros((2 * M * HEADS, 2 * HEADS), np.float32)
    for i in range(2):
        for m in range(M):
            for h in range(HEADS):
                selA[i * M * HEADS + m * HEADS + h, i * HEADS + h] = 1.0 / M
    rep8 = np.zeros((2 * M, 2 * M * HEADS), np.float32)
    for q in range(2 * M * HEADS):
        rep8[q // HEADS, q] = 1.0
    baseA = np.zeros((2 * M * HEADS, BPC), np.float32)
    for q in range(2 * M * HEADS):
        for b in range(BPC):
            baseA[q, b] = (b * HEADS + q % HEADS) * L + 1
    baseS = np.zeros((4 * M, 1), np.float32)
    for k in range(4 * M):
        baseS[k, 0] = (k // (2 * M)) * L + 1

    cidx = np.zeros((96, CIDX_COLS), np.float32)
    cidx[0:8, 0:96] = rep8
    cidx[0:96, 96:98] = baseA
    cidx[0:16, 98:99] = baseS
    cid2 = np.zeros((128, CID2_COLS), np.float32)
    cid2[0:128, 0:128] = np.eye(128)
    # bbc filled per-call (bias input)

    cb16 = np.zeros((128, CB16_COLS), ml_dtypes.bfloat16)
    for y in range(BLK):
        for p in range(128):
            cb16[(p // BLK) * BLK + y, RYS0 + y * 128 + p] = 1.0
    cb16[0:16, SELE0:SELE0 + 4] = selE
    cb16[0:96, SELA0:SELA0 + 24] = selA
    cb16[0:12, W120:W120 + 1] = 1.0 / HEADS
    cb16[0:1, SELBH0:SELBH0 + 4] = np.array([1.0, 0.0, 1.0, 0.0])
    cb16[0:1, SELBH0 + 4:SELBH0 + 8] = np.array([0.0, 1.0, 0.0, 1.0])

    perm = np.empty(EMB * BLK, np.int64)
    for cch in range(NEMB):
        for y in range(BLK):
            for p in range(128):
                g = cch * 16 + p // BLK
                x = p % BLK
                perm[(cch * BLK + y) * 128 + p] = g * 64 + x * BLK + y
    return {"cidx": cidx, "cid2": cid2, "cb16": cb16, "perm": perm}


def _build_program(stage=99):
    nc = bacc.Bacc("TRN2", target_bir_lowering=False, debug=False)

    seq_h = nc.dram_tensor("seq", [BPC * L, HID], BF16, kind="ExternalInput")
    attn_h = nc.dram_tensor("attn", [BPC * HEADS * L, L], BF16, kind="ExternalInput")
    pos_h = nc.dram_tensor("pos", [4 * M, 1], I32, kind="ExternalInput")
    posb_hs = [
        nc.dram_tensor(f"posb{b}", [2 * M, 1], I32, kind="ExternalInput")
        for b in range(BPC)
    ]
    ner_h = nc.dram_tensor("ner", [NER, 4], F32, kind="ExternalInput")
    whs_h = nc.dram_tensor("whs", [128, KCH * EMB + EMB], BF16, kind="ExternalInput")
    wts_h = nc.dram_tensor("wts", [128, KCH * EMB + EMB], BF16, kind="ExternalInput")
    wbs_h = nc.dram_tensor("wbs", [128, NBL * 128], BF16, kind="ExternalInput")
    cidx_h = nc.dram_tensor("cidx", [96, CIDX_COLS], F32, kind="ExternalInput")
    cid2_h = nc.dram_tensor("cid2", [128, CID2_COLS], F32, kind="ExternalInput")
    cb16_h = nc.dram_tensor("cb16", [128, CB16_COLS], BF16, kind="ExternalInput")
    out_h = nc.dram_tensor("logitsT", [NCLS, BPC], F32, kind="ExternalOutput")

    with TileContext(nc) as tc:
        with (
            tc.tile_pool(name="const", bufs=1) as cp,
            tc.tile_pool(name="data", bufs=1) as dp,
            tc.tile_pool(name="psbig", bufs=1, space="PSUM") as psb,
            tc.tile_pool(name="psea", bufs=2, space="PSUM") as pse,
            tc.tile_pool(name="pssm", bufs=3, space="PSUM") as pss,
        ):
            # ---- critical small loads first (sync queue) ----
            cidx = cp.tile([96, CIDX_COLS], F32)
            nc.sync.dma_start(cidx[:], cidx_h[:])
            posi = dp.tile([4 * M, 1], I32)
            pos_dma = nc.sync.dma_start(posi[:], pos_h[:])
            posbi = []
            for b in range(BPC):
                t = dp.tile([2 * M, 1], I32, tag=f"posbi{b}")
                nc.sync.dma_start(t[:], posb_hs[b][:])
                posbi.append(t)
            cb = cp.tile([128, CB16_COLS], BF16)
            nc.sync.dma_start(cb[:], cb16_h[:])
            cid2 = cp.tile([128, CID2_COLS], F32)
            nc.sync.dma_start(cid2[:], cid2_h[:])
            rep8 = cidx[0:8, 0:96]
            baseA = cidx[0:96, 96:98]
            baseS = cidx[0:16, 98:99]
            bbc = cid2[0:97, 128:129]
            rys = cb[:, RYS0:RYS0 + 1024]
            selE = cb[0:16, SELE0:SELE0 + 4]
            selA = cb[0:96, SELA0:SELA0 + 24]
            w12 = cb[0:12, W120:W120 + 1]
            bhr = cb[0:1, BHR0:BHR0 + EMB]
            btr = cb[0:1, BTR0:BTR0 + EMB]
            selbh = cb[0:1, SELBH0:SELBH0 + 4]
            selbt = cb[0:1, SELBH0 + 4:SELBH0 + 8]
            idf = cid2[:, 0:128]

            ner4f = dp.tile([NER, 4], F32)
            nc.sync.dma_start(ner4f[:], ner_h[:])
            ner4 = dp.tile([NER, 4], BF16)
            nc.vector.tensor_copy(ner4[:], ner4f[:])

            seqt = []
            for b in range(BPC):
                t = dp.tile([128, NL * HID], BF16, tag=f"seq{b}")
                nc.sync.dma_start(
                    t[:].rearrange("p (c d) -> p c d", c=NL),
                    seq_h[b * L:(b + 1) * L, :].rearrange("(c p) d -> p c d", p=128))
                seqt.append(t)

            # ---- bulk weight loads (scalar queue; host pre-rearranged).
            # Delay them behind the tiny index loads so the critical
            # gather path is not starved of SDMA bandwidth.
            from concourse.tile_rust import add_dep_helper
            whsf = cp.tile([128, KCH * EMB + EMB], BF16)
            d1 = nc.scalar.dma_start(whsf[:], whs_h[:])
            whs = whsf[:, 0:KCH * EMB]
            whn = whsf[0:NER, KCH * EMB:KCH * EMB + EMB]
            wtsf = cp.tile([128, KCH * EMB + EMB], BF16)
            d2 = nc.scalar.dma_start(wtsf[:], wts_h[:])
            wts = wtsf[:, 0:KCH * EMB]
            wtn = wtsf[0:NER, KCH * EMB:KCH * EMB + EMB]
            wbs = cp.tile([128, NBL * 128], BF16)
            d3 = nc.scalar.dma_start(wbs[:], wbs_h[:])
            for d in (d1, d2, d3):
                add_dep_helper(d.ins, pos_dma.ins,
                               reason="weights yield SDMA to index loads")

            # ---- index computation ----
            posf = dp.tile([4 * M, 1], F32)
            nc.vector.tensor_copy(posf[:], posi[:])
            idxsf = dp.tile([4 * M, 1], F32)
            nc.vector.tensor_add(idxsf[:], posf[:], baseS)
            idxs = dp.tile([4 * M, 1], I32)
            nc.vector.tensor_copy(idxs[:], idxsf[:])

            idxa = []
            for b in range(BPC):
                posbf = dp.tile([2 * M, 1], F32, tag=f"posbf{b}")
                nc.vector.tensor_copy(posbf[:], posbi[b][:])
                ps_idx = pss.tile([2 * M * HEADS, 1], F32, tag="sm")
                nc.tensor.matmul(ps_idx[:], lhsT=rep8, rhs=posbf[:],
                                 start=True, stop=True)
                idxaf = dp.tile([2 * M * HEADS, 1], F32, tag=f"idxaf{b}")
                nc.vector.tensor_add(idxaf[:], ps_idx[:], baseA[:, b:b + 1])
                ia = dp.tile([2 * M * HEADS, 1], I32, tag=f"idxa{b}")
                nc.vector.tensor_copy(ia[:], idxaf[:])
                idxa.append(ia)

            # ---- gathers ----
            sg = dp.tile([4 * M, HID], BF16)
            nc.gpsimd.indirect_dma_start(
                out=sg[:], out_offset=None, in_=seq_h[:],
                in_offset=bass.IndirectOffsetOnAxis(ap=idxs[:, :1], axis=0))
            at = []
            for b in range(BPC):
                t = dp.tile([2 * M * HEADS, L], BF16, tag=f"at{b}")
                nc.gpsimd.indirect_dma_start(
                    out=t[:], out_offset=None, in_=attn_h[:],
                    in_offset=bass.IndirectOffsetOnAxis(ap=idxa[b][:, :1], axis=0))
                at.append(t)

            if stage < 1:
                lg = dp.tile([NCLS, BPC], F32)
                nc.vector.memset(lg[:], 0.0)
                nc.vector.tensor_copy(lg[0:4 * M, 0:1], sg[:, 0:1])
                nc.vector.tensor_copy(lg[0:2 * M * HEADS - 31, 1:2], at[0][0:2 * M * HEADS - 31, 0:1])
                nc.sync.dma_start(out_h[:], lg[:])
                return _finish(nc)

            # ---- entity embeddings: log-sum-exp over mentions ----
            exps = dp.tile([4 * M, HID], BF16)
            nc.scalar.activation(exps[:], sg[:], AF.Exp)
            ps_e = psb.tile([4, HID], F32, tag="big")
            for n0, nl_ in ((0, 512), (512, 256)):
                nc.tensor.matmul(ps_e[:, n0:n0 + nl_], lhsT=selE,
                                 rhs=exps[:, n0:n0 + nl_], start=True, stop=True)
            ent = dp.tile([4, HID], F32)
            nc.scalar.activation(ent[:], ps_e[:], AF.Ln)
            ps_et = pss.tile([128, 4 * NEMB], F32, tag="sm")
            for c in range(NEMB):
                nc.tensor.transpose(ps_et[:, c * 4:(c + 1) * 4],
                                    ent[:, c * 128:(c + 1) * 128], idf[0:4, 0:4])
            entT = dp.tile([128, 4 * NEMB], BF16)
            nc.vector.tensor_copy(entT[:], ps_et[:])

            # ---- entity attention pooling + context vector ----
            htc = []
            for b in range(BPC):
                ps_eah = pse.tile([HEADS, L], F32, tag="ea")
                nc.tensor.matmul(ps_eah[:], lhsT=selA[:, 0:HEADS], rhs=at[b][:],
                                 start=True, stop=True)
                ps_eat = pse.tile([HEADS, L], F32, tag="ea")
                nc.tensor.matmul(ps_eat[:], lhsT=selA[:, HEADS:2 * HEADS],
                                 rhs=at[b][:], start=True, stop=True)
                eah = dp.tile([HEADS, L], F32, tag=f"eah{b}")
                nc.vector.tensor_copy(eah[:], ps_eah[:])
                prd = dp.tile([HEADS, L], BF16, tag=f"prd{b}")
                nc.vector.tensor_tensor(out=prd[:], in0=eah[:], in1=ps_eat[:],
                                        op=ALU.mult)
                ps_ht = pss.tile([1, L], F32, tag="sm")
                nc.tensor.matmul(ps_ht[:], lhsT=w12, rhs=prd[:],
                                 start=True, stop=True)
                sm = dp.tile([1, 1], F32, tag=f"sm{b}")
                nc.vector.reduce_sum(sm[:], ps_ht[:], axis=mybir.AxisListType.X)
                den = dp.tile([1, 1], F32, tag=f"den{b}")
                nc.vector.tensor_scalar_add(den[:], sm[:], 1e-5)
                rcp = dp.tile([1, 1], F32, tag=f"rcp{b}")
                nc.vector.reciprocal(rcp[:], den[:])
                htn = dp.tile([1, L], F32, tag=f"htn{b}")
                nc.vector.tensor_scalar_mul(htn[:], ps_ht[:], rcp[:, :1])
                ps_htc = pss.tile([128, NL], F32, tag="sm")
                for c in range(NL):
                    nc.tensor.transpose(ps_htc[:, c:c + 1],
                                        htn[:, c * 128:(c + 1) * 128],
                                        idf[0:1, 0:1])
                h = dp.tile([128, NL], BF16, tag=f"htc{b}")
                nc.vector.tensor_copy(h[:], ps_htc[:])
                htc.append(h)

            if stage < 2:
                lg = dp.tile([NCLS, BPC], F32)
                nc.vector.memset(lg[:], 0.0)
                nc.vector.tensor_copy(lg[0:97, 0:1], entT[0:97, 0:1])
                nc.vector.tensor_copy(lg[0:97, 1:2], htc[0][0:97, 0:1])
                nc.sync.dma_start(out_h[:], lg[:])
                return _finish(nc)

            # ---- rs = seq^T @ ht  (column form) ----
            ps_rsc = pss.tile([128, NEMB * BPC], F32, tag="sm")
            for b in range(BPC):
                for d in range(NEMB):
                    for c in range(NL):
                        nc.tensor.matmul(
                            ps_rsc[:, d * BPC + b:d * BPC + b + 1],
                            lhsT=seqt[b][:, c * HID + d * 128:c * HID + (d + 1) * 128],
                            rhs=htc[b][:, c:c + 1],
                            start=(c == 0), stop=(c == NL - 1))
            rsc = dp.tile([128, 4 * NEMB], BF16)
            nc.vector.tensor_copy(
                rsc[:].rearrange("p (r b m) -> p r b m", r=NEMB, b=BPC),
                ps_rsc[:].rearrange("p (r b) -> p r b", r=NEMB)
                .unsqueeze(3).broadcast_to([128, NEMB, BPC, 2]))

            # ---- extractor GEMVs:  [4,768] = cat4^T @ W^T  ----
            def cat_chunk(j):
                if j < NEMB:
                    return entT[:, j * 4:(j + 1) * 4]
                if j < 2 * NEMB:
                    return rsc[:, (j - NEMB) * 4:(j - NEMB + 1) * 4]
                return ner4[:]

            t4 = []
            for wi, (ws, wn, selb, br) in enumerate(
                    ((whs, whn, selbh, bhr), (wts, wtn, selbt, btr))):
                ps_w = psb.tile([4, EMB], F32, tag="big")
                for n0, nl_ in ((0, 512), (512, 256)):
                    for j in range(KCH + 1):
                        lhsT = cat_chunk(j)
                        rhs = (ws[:, j * EMB + n0:j * EMB + n0 + nl_] if j < KCH
                               else wn[:, n0:n0 + nl_])
                        nc.tensor.matmul(ps_w[:, n0:n0 + nl_], lhsT=lhsT, rhs=rhs,
                                         start=(j == 0), stop=False)
                    nc.tensor.matmul(ps_w[:, n0:n0 + nl_], lhsT=selb,
                                     rhs=br[:, n0:n0 + nl_], start=False, stop=True)
                t = dp.tile([4, EMB], F32, tag=f"t4_{wi}")
                nc.scalar.activation(t[:], ps_w[:], AF.Tanh)
                t4.append(t)

            if stage < 3:
                lg = dp.tile([NCLS, BPC], F32)
                nc.vector.memset(lg[:], 0.0)
                nc.vector.tensor_copy(lg[0:4, 0:2], t4[0][:, 0:2])
                nc.vector.tensor_copy(lg[0:4, 1:2], t4[1][:, 0:1])
                nc.vector.tensor_copy(lg[0:89, 0:1], rsc[0:89, 0:1])
                nc.sync.dma_start(out_h[:], lg[:])
                return _finish(nc)

            # ---- transpose hs2/ts2 to columns ----
            ps_a = pss.tile([128, 4 * NEMB], F32, tag="sm")
            ps_b2 = pss.tile([128, 4 * NEMB], F32, tag="sm")
            for c in range(NEMB):
                nc.tensor.transpose(ps_a[:, c * 4:(c + 1) * 4],
                                    t4[0][:, c * 128:(c + 1) * 128], idf[0:4, 0:4])
                nc.tensor.transpose(ps_b2[:, c * 4:(c + 1) * 4],
                                    t4[1][:, c * 128:(c + 1) * 128], idf[0:4, 0:4])
            h2t = dp.tile([128, 4 * NEMB], BF16)
            nc.vector.tensor_copy(
                h2t[:].rearrange("p (c b) -> p c b", c=NEMB)[:, :, 0:4:2],
                ps_a[:].rearrange("p (c b) -> p c b", c=NEMB)[:, :, 0:4:2])
            nc.vector.tensor_copy(
                h2t[:].rearrange("p (c b) -> p c b", c=NEMB)[:, :, 1:4:2],
                ps_b2[:].rearrange("p (c b) -> p c b", c=NEMB)[:, :, 1:4:2])

            if stage < 4:
                lg = dp.tile([NCLS, BPC], F32)
                nc.vector.memset(lg[:], 0.0)
                nc.vector.tensor_copy(lg[0:97, 0:2], h2t[0:97, 0:2])
                nc.sync.dma_start(out_h[:], lg[:])
                return _finish(nc)

            # ---- grouped bilinear + classifier ----
            # ts-replication: out col layout (y, c, b) = y*12 + c*2 + b
            ps_t2x = pss.tile([128, BLK * NEMB * BPC], F32, tag="sm")
            tscols = h2t[:].rearrange("p (c b) -> p c b", c=NEMB)[:, :, 1:4:2]
            for y in range(BLK):
                nc.tensor.matmul(
                    ps_t2x[:, y * 12:(y + 1) * 12]
                    .rearrange("p (c b) -> p c b", c=NEMB),
                    lhsT=rys[:, y * 128:(y + 1) * 128],
                    rhs=tscols, start=True, stop=True)
            if stage < 5:
                lg = dp.tile([NCLS, BPC], F32)
                nc.vector.memset(lg[:], 0.0)
                nc.vector.tensor_copy(lg[0:97, 0:2], ps_t2x[0:97, 0:2])
                nc.sync.dma_start(out_h[:], lg[:])
                return _finish(nc)

            blt = dp.tile([128, NEMB * 16], BF16)
            for c in range(NEMB):
                nc.vector.tensor_tensor(
                    out=blt[:, c * 16:(c + 1) * 16]
                    .rearrange("p (y b) -> p y b", y=BLK),
                    in0=h2t[:, c * 4:c * 4 + 4:2].unsqueeze(1)
                        .broadcast_to([128, BLK, 2]),
                    in1=ps_t2x[:].rearrange("p (y c b) -> p y c b", y=BLK, c=NEMB)
                    [:, :, c, :],
                    op=ALU.mult)
            if stage < 6:
                lg = dp.tile([NCLS, BPC], F32)
                nc.vector.memset(lg[:], 0.0)
                nc.vector.tensor_copy(lg[0:97, 0:2], blt[0:97, 0:2])
                nc.sync.dma_start(out_h[:], lg[:])
                return _finish(nc)

            ps_l = pss.tile([NCLS, BPC], F32, tag="sm")
            for c in range(NEMB):
                for y in range(BLK):
                    k = c * BLK + y
                    nc.tensor.matmul(ps_l[:], lhsT=wbs[:, k * 128:k * 128 + NCLS],
                                     rhs=blt[:, c * 16 + y * 2:c * 16 + y * 2 + 2],
                                     start=(k == 0), stop=(k == NBL - 1))
            lg = dp.tile([NCLS, BPC], F32)
            if stage < 7:
                nc.vector.memset(lg[:], 0.0)
                nc.vector.tensor_copy(lg[0:1, 0:1], ps_l[0:1, 0:1])
            else:
                nc.vector.tensor_scalar_add(lg[:], ps_l[:], bbc[:, :1])
            nc.sync.dma_start(out_h[:], lg[:])

    return _finish(nc)


def _finish(nc):
    return nc


def _get_program():
    if "nc" not in _cache:
        nc = _build_program()
        nc.finalize()
        _cache["nc"] = nc
        _cache["consts"] = _build_constants()
    return _cache["nc"], _cache["consts"]


def kernel(sequence_output, attention, entity_pos, hs_ner_tags, ts_ner_tags,
           Wh, bh, Wt, bt, Wb, bb):
    nc, c = _get_program()

    seq = np.asarray(sequence_output, dtype=np.float32).astype(ml_dtypes.bfloat16)
    attn = np.asarray(attention, dtype=np.float32).astype(ml_dtypes.bfloat16)
    pos = np.asarray(entity_pos).astype(np.int32)
    nh = np.asarray(hs_ner_tags, dtype=np.float32)
    nt = np.asarray(ts_ner_tags, dtype=np.float32)
    whT = np.ascontiguousarray(np.asarray(Wh, dtype=np.float32).T).astype(ml_dtypes.bfloat16)
    wtT = np.ascontiguousarray(np.asarray(Wt, dtype=np.float32).T).astype(ml_dtypes.bfloat16)
    wbT = np.ascontiguousarray(np.asarray(Wb, dtype=np.float32).T)[c["perm"]]
    wbT = np.pad(wbT, ((0, 0), (0, 128 - NCLS))).astype(ml_dtypes.bfloat16)

    def sbuf_image(w, extra):
        main = w[0:KCH * 128].reshape(KCH, 128, EMB).transpose(1, 0, 2).reshape(128, KCH * EMB)
        img = np.zeros((128, KCH * EMB + EMB), ml_dtypes.bfloat16)
        img[:, 0:KCH * EMB] = main
        img[0:NER, KCH * EMB:] = extra
        return img

    whs = sbuf_image(whT, whT[KCH * 128:CAT])
    wts = sbuf_image(wtT, wtT[KCH * 128:CAT])
    wbs = wbT.reshape(NBL, 128, 128).transpose(1, 0, 2).reshape(128, NBL * 128)
    wbs = np.ascontiguousarray(wbs)

    cb16 = c["cb16"].copy()
    cb16[0:1, BHR0:BHR0 + EMB] = np.asarray(bh, np.float32).reshape(1, EMB)
    cb16[32:33, BHR0:BHR0 + EMB] = np.asarray(bt, np.float32).reshape(1, EMB)
    cid2 = c["cid2"].copy()
    cid2[0:97, 128] = np.asarray(bb, np.float32)

    in_maps = []
    for core in range(NCORES):
        b0 = core * BPC
        pc = np.ascontiguousarray(pos[b0:b0 + BPC])          # [2,2,M]
        ner = np.stack([nh[b0], nt[b0], nh[b0 + 1], nt[b0 + 1]], axis=1)
        im = {
            "seq": np.ascontiguousarray(seq[b0:b0 + BPC]).reshape(BPC * L, HID),
            "attn": np.ascontiguousarray(attn[b0:b0 + BPC]).reshape(BPC * HEADS * L, L),
            "pos": pc.reshape(4 * M, 1),
            "ner": np.ascontiguousarray(ner.astype(np.float32)),
            "whs": whs, "wts": wts, "wbs": wbs,
            "cidx": c["cidx"], "cid2": cid2, "cb16": cb16,
        }
        for b in range(BPC):
            im[f"posb{b}"] = np.ascontiguousarray(pc[b].reshape(2 * M, 1))
        in_maps.append(im)

    res = run_bass_kernel_spmd(nc, in_maps, core_ids=list(range(NCORES)))
    _cache["last_res"] = res
    out = np.empty((B, NCLS), np.float32)
    for core in range(NCORES):
        out[core * BPC:(core + 1) * BPC] = res.results[core]["logitsT"].T
    return out
